# revision 42
# baseline (speedup 1.0000x reference)
"""Trainium2 Bass kernel for per-pixel dynamic 3D filtering.

    out[b, h, w, o] = sum_k patches[b, h, w, k] * f[b, h, w, k, o]

with patches = im2col(x) over a 3x3 spatial window (zero-padded SAME) and
3 time steps, k ordered (kh, kw, t), K=27, C_out=16, B=8, H=W=192.

Sharding: pure data parallel over batch — core c computes image c.

Per-core device layout (one image):
  * pixels are mapped to SBUF partitions in 8h x 16w blocks: a "supertile"
    covers 8 image rows x all 192 columns; partition p = dh*16 + dw holds the
    12 consecutive pixels w in [dw*12, dw*12+12).  With this mapping, the
    f-slab of a supertile is EXACTLY a contiguous row-major [128, 5184] slice
    of f, and the out-slab is a contiguous [128, 192] slice of out.
  * im2col of the small x tensor is done host-side and uploaded pre-blocked
    (4MB vs f's 64MB) — generating it on-device would cost more HBM traffic
    than uploading it.

Compute (v2 wide-scan, default): ONE custom DVE instruction per supertile.
The op DYNF_MAC_SCAN_ANT computes scan(ADD, Src0*Src1) — a running prefix
sum of the patch*f products — so every f element is touched exactly once
(vs two passes for the stock tensor_tensor + tensor_reduce pair, the v1
fallback). The key AP trick: the per-pixel stride is 432 = 27*16, so for a
fixed output channel o the whole supertile's (pixel, tap) stream is ONE
affine dim (step 16, count 324); in0 = [P, o:16 step 1, gk:324 step 16]
covers all 12 pixel groups in a single instruction. The prefix is stored
linearly in stream order with a zeroed pad element in front; every
(pixel, o) tap-sum is then prefix[end_i] - prefix[end_i - 27] — uniform
across pixel AND o-row boundaries — recovered by ONE strided tensor_sub.
DVE per supertile: 1 scan (FD 5184) + 1 sub (FD 192); the pad zeroing
rides the idle ACT engine.

Pipelining: f-slab DMA split in two halves on the sync-engine HWDGE ring
(kept as a pure prefetch stream); out-DMAs ride the ACT HWDGE ring so a
sem-waiting store can never stall the next f load; fbufs=3/prefbufs=3/
obufs=6 buffering.

Measured (8 cores concurrent, per-iteration steady state via the
(T(17)-T(1))/16 NEFF-repetition method): ~191-192us vs a ~188-199us
pure-DMA floor for the same 70MB/core traffic — at the HBM memory
roofline. (Stock-op v1: ~310us; 12-scans-per-supertile v2: ~205-230us.)
"""

import os
from contextlib import ExitStack

import numpy as np

# ---- problem constants (hardcoded per contract) ---------------------------
B, T, H, W = 8, 3, 192, 192
K = 3
PAD = K // 2
KK = T * K * K  # 27
CO = 16
N_CORES = 8

# supertile geometry
DH, DW, G = 8, 16, 12  # partitions = DH*DW = 128; per-partition pixels = G
P = DH * DW  # 128
N_ST = H // DH  # 24 supertiles per image
FFREE = G * KK * CO  # 5184 f32 per partition per supertile
PFREE = G * KK  # 324 patch f32 per partition per supertile
OFREE = G * CO  # 192 out f32 per partition per supertile


def _im2col_batch(x: np.ndarray) -> np.ndarray:
    """x: (B, T, H, W) f32 -> patches (B, H, W, 27), k ordered (kh, kw, t)."""
    Bb, Tt, Hh, Ww = x.shape
    xp = np.pad(x, ((0, 0), (0, 0), (PAD, PAD), (PAD, PAD)))
    cols = [
        xp[:, t, i : i + Hh, j : j + Ww]
        for i in range(K)
        for j in range(K)
        for t in range(Tt)
    ]
    return np.stack(cols, axis=-1).astype(np.float32)


XFREE = T * K * 16  # 144: per-partition per-supertile x-window (wl padded 14->16)


def _xpp_batch(x: np.ndarray) -> np.ndarray:
    """Per-partition x windows: (B,T,H,W) -> (B, N_ST*P, 144), layout
    (t, kh, wl) per partition; value = xp[t, 8s+dh+kh, dw*12+wl], wl<14."""
    xp = np.pad(x, ((0, 0), (0, 0), (PAD, PAD), (PAD, PAD))).astype(np.float32)
    out = np.zeros((x.shape[0], N_ST, DH, DW, T, K, 16), np.float32)
    rows = np.arange(H).reshape(N_ST, DH)
    cols = (np.arange(DW) * G)[:, None] + np.arange(14)[None, :]
    for kh in range(K):
        sub = xp[:, :, rows + kh, :][:, :, :, :, cols]  # (B,T,NST,DH,DW,14)
        out[..., kh, :14] = np.moveaxis(sub, 1, 4)
    return out.reshape(x.shape[0], N_ST * P, XFREE)


def _register_custom_op():
    """Register DYNF_MAC_SCAN_ANT: out = running_sum(in0 * in1) along the free
    stream (inclusive prefix scan of the product). One DVE pass fuses the
    multiply and the k-reduction; segment sums fall out as differences of the
    prefix at segment-end positions."""
    import concourse.dve_ops as dve_ops
    from concourse.dve_spec import AluOp, Spec, Src0, Src1, _has_src1, lower, scan
    from concourse.dve_uop import DveOpSpec

    name = "DYNF_MAC_SCAN_ANT"
    for op in dve_ops.OPS:
        if op.name == name:
            return op

    def _ref(in0, in1, c0, c1, c2):
        prod = np.asarray(in0, np.float32) * np.asarray(in1, np.float32)
        flat = prod.reshape(prod.shape[0], -1)
        return np.cumsum(flat, axis=1, dtype=np.float32).reshape(prod.shape)

    spec = Spec(body=scan(AluOp.ADD, Src0 * Src1), reference=_ref)
    row = dve_ops._CUSTOM_DVE_ROW_BASE + len(dve_ops.OPS)
    assert row < 0x20
    shas = {}
    for ver in ("v3", "v4"):
        s = DveOpSpec(
            name=name, opcode=row, uops=lower(spec, ver=ver), rd1_en=_has_src1(spec)
        )
        shas[ver] = s.sha(ver)
    op = dve_ops.DveOp(name, spec, subdim=False, uops_sha=shas)
    dve_ops.OPS.append(op)
    dve_ops._SUB_OPCODE_FOR_NAME[name] = row
    dve_ops.CUSTOM_DVE_SPECS[name] = spec
    return op


def _build_program_v2(reps: int = 1, mode: str = "full"):
    """v2: fused multiply+scan custom DVE op — one DVE pass over f instead of
    two (tensor_tensor mult + tensor_reduce).

    mode: "full" | "dma" (no compute) | "scan" (no extraction) — diagnostics."""
    import concourse.bacc as bacc
    import concourse.tile as tile
    from concourse import mybir

    f32 = mybir.dt.float32
    mac_op = _register_custom_op()
    patch_mode = os.environ.get("DYNF_PATCH_MODE", "packed")

    nc = bacc.Bacc("TRN2", debug=False, enable_asserts=False)

    f_ap = nc.dram_tensor("f_in", (N_ST * P, FFREE), f32, kind="ExternalInput").ap()
    if patch_mode == "expand":
        p_ap = nc.dram_tensor(
            "p_in", (N_ST * P, XFREE), f32, kind="ExternalInput"
        ).ap()
    else:
        p_ap = nc.dram_tensor(
            "p_in", (N_ST * P, PFREE), f32, kind="ExternalInput"
        ).ap()
    o_ap = nc.dram_tensor("o_out", (N_ST * P, OFREE), f32, kind="ExternalOutput").ap()

    fbufs = int(os.environ.get("DYNF_FBUFS", "3"))
    prefbufs = int(os.environ.get("DYNF_PREFBUFS", "3"))
    obufs = int(os.environ.get("DYNF_OBUFS", "6"))
    # default: extraction on DVE. gpsimd-extraction measured faster once but
    # produced NRT_EXEC_UNIT_UNRECOVERABLE device crashes when combined with
    # split f-DMAs — not worth the risk.
    ext_eng = os.environ.get("DYNF_EXT_ENGINE", "vector")

    with tile.TileContext(nc) as tc, ExitStack() as ctx:
        fpool = ctx.enter_context(tc.tile_pool(name="fpool", bufs=fbufs))
        ppool = ctx.enter_context(tc.tile_pool(name="ppool", bufs=3))
        prefpool = ctx.enter_context(tc.tile_pool(name="prefpool", bufs=prefbufs))
        opool = ctx.enter_context(tc.tile_pool(name="opool", bufs=obufs))

        zpool = ctx.enter_context(tc.tile_pool(name="zpool", bufs=1))
        zerot = zpool.tile([P, 1], f32)
        nc.vector.memset(zerot[:], 0.0)

        if mode == "dve":
            # pure DVE throughput probe: one resident f/p tile, all scans
            ft0 = fpool.tile([P, FFREE], f32)
            nc.sync.dma_start(ft0[:], f_ap[0:P, :])
            pt0 = ppool.tile([P, PFREE], f32, tag="pt")
            nc.sync.dma_start(pt0[:], p_ap[0:P, :])
            for _ in range(reps):
                for s in range(N_ST):
                    rows = slice(s * P, (s + 1) * P)
                    pref = prefpool.tile([P, FFREE], f32)
                    for g in range(G):
                        f_ok = ft0[:, g * KK * CO : (g + 1) * KK * CO].rearrange(
                            "p (k o) -> p o k", k=KK, o=CO
                        )
                        p_ok = (
                            pt0[:, g * KK : (g + 1) * KK]
                            .unsqueeze(1)
                            .broadcast_to([P, CO, KK])
                        )
                        pr_ok = pref[
                            :, g * KK * CO : (g + 1) * KK * CO
                        ].rearrange("p (o k) -> p o k", o=CO, k=KK)
                        nc.vector._custom_dve(
                            mac_op, out=pr_ok, in0=f_ok, in1=p_ok
                        )
                    nc.scalar.dma_start(o_ap[rows, :], pref[:, :OFREE])
            nc.compile()
            return nc

        for _ in range(reps):
            for s in range(N_ST):
                rows = slice(s * P, (s + 1) * P)
                ft = fpool.tile([P, FFREE], f32)
                nsplit = int(os.environ.get("DYNF_SPLIT", "2"))
                hw_elems = FFREE // nsplit
                for h in range(nsplit):
                    nc.sync.dma_start(
                        ft[:, h * hw_elems : (h + 1) * hw_elems],
                        f_ap[rows, h * hw_elems : (h + 1) * hw_elems],
                    )
                if patch_mode == "expand":
                    xt = ppool.tile([P, XFREE], f32, tag="xt")
                    nc.sync.dma_start(xt[:], p_ap[rows, :])
                    # expand windows -> patches on GPSIMD (idle engine):
                    # pt[g, kh, kw, t] = xt[t, kh, g+kw]
                    pt = ppool.tile([P, PFREE], f32, tag="pt")
                    pt5 = pt[:].rearrange(
                        "p (g kh kw t) -> p kh g kw t", g=G, kh=K, kw=K, t=T
                    )
                    xta = xt[:]
                    APc = type(xta)
                    exp_name = os.environ.get("DYNF_EXPAND_ENGINE", "scalar")
                    for kh in range(K):
                        src = APc(
                            xta.tensor,
                            xta.offset + kh * 16,
                            [list(xta.ap[0]), [1, G], [1, K], [K * 16, T]],
                        )
                        if exp_name == "scalar":
                            nc.scalar.copy(pt5[:, kh], src)
                        elif exp_name == "gpsimd":
                            nc.gpsimd.tensor_copy(pt5[:, kh], src)
                        else:
                            nc.vector.tensor_copy(pt5[:, kh], src)
                else:
                    pt = ppool.tile([P, PFREE], f32, tag="pt")
                    nc.sync.dma_start(pt[:], p_ap[rows, :])

                if mode == "dma":
                    nc.scalar.dma_start(o_ap[rows, :], ft[:, :OFREE])
                    continue

                if os.environ.get("DYNF_SCAN_WIDE", "1") == "1":
                    # ONE scan per supertile: for fixed o, addr(g,k) =
                    # (g*27+k)*16 + o is a single affine dim (432 == 27*16),
                    # so in0 = [P, o:16 step 1, gk:324 step 16] covers all 12
                    # pixel groups. Prefix stored linearly in stream order
                    # (offset 1; [0] is a pad so the i=0 difference stays
                    # in-tile); segment ends sit exactly 27 apart, so ONE
                    # tensor_sub recovers every segment sum — the -27
                    # neighbour is correct even across o-row boundaries.
                    pref = prefpool.tile([P, FFREE + 1], f32)
                    # zero the pad so the i=0 difference is E0 - 0. On DVE by
                    # default: an ACT-side copy would sit on the ACT queue
                    # ahead of out-DMAs carrying a pref-slot dependency.
                    if os.environ.get("DYNF_PAD_ENGINE", "scalar") == "scalar":
                        nc.scalar.copy(pref[:, 0:1], zerot[:])
                    else:
                        nc.vector.memset(pref[:, 0:1], 0.0)
                    APc = type(ft[:])
                    fa, pa, pra = ft[:], pt[:], pref[:]
                    GK = G * KK  # 324
                    in0 = APc(
                        fa.tensor, fa.offset, [list(fa.ap[0]), [1, CO], [CO, GK]]
                    )
                    in1 = APc(
                        pa.tensor, pa.offset, [list(pa.ap[0]), [0, CO], [1, GK]]
                    )
                    outp = APc(
                        pra.tensor,
                        pra.offset + 1,
                        [list(pra.ap[0]), [GK, CO], [1, GK]],
                    )
                    nc.vector._custom_dve(mac_op, out=outp, in0=in0, in1=in1)

                    if mode == "scan":
                        nc.scalar.dma_start(o_ap[rows, :], pref[:, :OFREE])
                        continue

                    ot = opool.tile([P, OFREE], f32)
                    oa = ot[:]
                    sub_out = APc(
                        oa.tensor, oa.offset, [list(oa.ap[0]), [1, CO], [CO, G]]
                    )
                    e1 = APc(
                        pra.tensor,
                        pra.offset + KK,
                        [list(pra.ap[0]), [GK, CO], [KK, G]],
                    )
                    e0 = APc(
                        pra.tensor, pra.offset, [list(pra.ap[0]), [GK, CO], [KK, G]]
                    )
                    eng = nc.gpsimd if ext_eng == "gpsimd" else nc.vector
                    eng.tensor_sub(sub_out, e1, e0)
                    if os.environ.get("DYNF_OUT_ENGINE", "scalar") == "sync":
                        nc.sync.dma_start(o_ap[rows, :], ot[:])
                    else:
                        nc.scalar.dma_start(o_ap[rows, :], ot[:])
                    continue

                ends_direct = os.environ.get("DYNF_ENDS_DIRECT", "0") == "1"
                if ends_direct:
                    # scans write through a step-0 (last-wins) out AP: only
                    # each segment's final prefix value survives, landing in a
                    # compact [P, G*CO] ends tile. No prefix buffer at all.
                    endst = prefpool.tile([P, OFREE], f32)
                    APc = type(ft[:])
                    ea = endst[:]
                    for g in range(G):
                        f_ok = ft[:, g * KK * CO : (g + 1) * KK * CO].rearrange(
                            "p (k o) -> p o k", k=KK, o=CO
                        )
                        p_ok = (
                            pt[:, g * KK : (g + 1) * KK]
                            .unsqueeze(1)
                            .broadcast_to([P, CO, KK])
                        )
                        e_ok = APc(
                            ea.tensor,
                            ea.offset + g * CO,
                            [list(ea.ap[0]), [1, CO], [0, KK]],
                        )
                        nc.vector._custom_dve(mac_op, out=e_ok, in0=f_ok, in1=p_ok)
                    ends = ea.rearrange("p (g o) -> p g o", g=G, o=CO)
                    if mode == "scan":
                        nc.scalar.dma_start(o_ap[rows, :], endst[:])
                        continue
                else:
                    # prefix sums of products, (o, k)-major per pixel slot
                    pref = prefpool.tile([P, FFREE], f32)
                    for g in range(G):
                        f_ok = ft[:, g * KK * CO : (g + 1) * KK * CO].rearrange(
                            "p (k o) -> p o k", k=KK, o=CO
                        )
                        p_ok = (
                            pt[:, g * KK : (g + 1) * KK]
                            .unsqueeze(1)
                            .broadcast_to([P, CO, KK])
                        )
                        pr_ok = pref[
                            :, g * KK * CO : (g + 1) * KK * CO
                        ].rearrange("p (o k) -> p o k", o=CO, k=KK)
                        nc.vector._custom_dve(mac_op, out=pr_ok, in0=f_ok, in1=p_ok)

                    if mode == "scan":
                        nc.scalar.dma_start(o_ap[rows, :], pref[:, :OFREE])
                        continue

                    pref4 = pref[:].rearrange(
                        "p (g o k) -> p g o k", g=G, o=CO, k=KK
                    )
                    ends = pref4[:, :, :, KK - 1 : KK].squeeze(3)  # [P, G, CO]

                # segment sums = differences of prefix at k = KK-1 positions
                ot = opool.tile([P, OFREE], f32)
                ot3 = ot[:].rearrange("p (g o) -> p g o", g=G, o=CO)
                eng = nc.gpsimd if ext_eng == "gpsimd" else nc.vector
                # the 1-input o=0 copy rides the otherwise-idle ACT engine
                nc.scalar.copy(ot3[:, :, 0:1], ends[:, :, 0:1])
                eng.tensor_sub(
                    ot3[:, :, 1:CO], ends[:, :, 1:CO], ends[:, :, 0 : CO - 1]
                )

                # out-DMA on the ACT HWDGE ring: keeps the sync-engine ring a
                # pure f/p prefetch stream (a sem-waiting out-DMA on the same
                # FIFO would stall the next supertile's f load).
                if mode == "ext":
                    nc.scalar.dma_start(o_ap[rows, :], ft[:, :OFREE])
                else:
                    nc.scalar.dma_start(o_ap[rows, :], ot[:])

    nc.compile()
    return nc


def _build_program(reps: int = 1):
    """Build the Bass/Tile program once; returns nc.

    reps > 1 repeats the whole per-image computation (benchmark variant:
    dispatch overhead cancels in (T(reps) - T(1)) / (reps - 1))."""
    import concourse.bacc as bacc
    import concourse.tile as tile
    from concourse import mybir

    f32 = mybir.dt.float32

    nc = bacc.Bacc("TRN2", debug=False, enable_asserts=False)

    f_ap = nc.dram_tensor("f_in", (N_ST * P, FFREE), f32, kind="ExternalInput").ap()
    p_ap = nc.dram_tensor("p_in", (N_ST * P, PFREE), f32, kind="ExternalInput").ap()
    o_ap = nc.dram_tensor("o_out", (N_ST * P, OFREE), f32, kind="ExternalOutput").ap()

    with tile.TileContext(nc) as tc, ExitStack() as ctx:
        fpool = ctx.enter_context(tc.tile_pool(name="fpool", bufs=3))
        ppool = ctx.enter_context(tc.tile_pool(name="ppool", bufs=3))
        prodpool = ctx.enter_context(tc.tile_pool(name="prodpool", bufs=2))
        opool = ctx.enter_context(tc.tile_pool(name="opool", bufs=3))

        for _ in range(reps):
            for s in range(N_ST):
                rows = slice(s * P, (s + 1) * P)
                ft = fpool.tile([P, FFREE], f32)
                nc.sync.dma_start(ft[:], f_ap[rows, :])
                pt = ppool.tile([P, PFREE], f32)
                nc.sync.dma_start(pt[:], p_ap[rows, :])

                # products: [128, (g, k, o)] = f * patches (broadcast on o)
                prod = prodpool.tile([P, FFREE], f32)
                f_gko = ft[:].rearrange("p (g k o) -> p g k o", g=G, k=KK, o=CO)
                p_gk1 = (
                    pt[:]
                    .rearrange("p (g k) -> p g k", g=G, k=KK)
                    .unsqueeze(3)
                    .broadcast_to([P, G, KK, CO])
                )
                prod_gko = prod[:].rearrange(
                    "p (g k o) -> p g k o", g=G, k=KK, o=CO
                )
                nc.vector.tensor_tensor(prod_gko, f_gko, p_gk1, mybir.AluOpType.mult)

                # reduce over k (innermost axis of the presented AP)
                ot = opool.tile([P, OFREE], f32)
                prod_gok = prod[:].rearrange("p (g k o) -> p g o k", g=G, k=KK, o=CO)
                ot_go = ot[:].rearrange("p (g o) -> p g o", g=G, o=CO)
                nc.vector.tensor_reduce(
                    ot_go, prod_gok, mybir.AxisListType.X, mybir.AluOpType.add
                )

                nc.sync.dma_start(o_ap[rows, :], ot[:])

    nc.compile()
    return nc


_NC_CACHE = None

# test harness introspection: last BassKernelResults (exec_time_ns when traced)
LAST_RESULTS = None


def build_program(reps: int = 1):
    ver = os.environ.get("DYNF_KERNEL_VERSION", "2")
    if ver == "2":
        try:
            return _build_program_v2(reps)
        except Exception:
            # custom-DVE registration/lowering failed (e.g. concourse drift):
            # fall back to the stock-op kernel (slower but correct). Flag the
            # fallback so prepare_in_maps stages the matching p_in layout.
            os.environ["DYNF_KERNEL_VERSION"] = "1"
            os.environ.pop("DYNF_PATCH_MODE", None)
    return _build_program(reps)


def _get_nc():
    global _NC_CACHE
    if _NC_CACHE is None:
        _NC_CACHE = build_program(1)
    return _NC_CACHE


def prepare_in_maps(x: np.ndarray, f: np.ndarray) -> list[dict]:
    """Host-side staging: per-core {f_in, p_in} in the device DRAM layouts."""
    x = np.asarray(x, dtype=np.float32)
    f = np.asarray(f, dtype=np.float32)
    assert x.shape == (B, T, H, W) and f.shape == (B, H, W, KK, CO)

    if os.environ.get("DYNF_PATCH_MODE", "packed") == "expand":
        p_blk = _xpp_batch(x)  # (B, N_ST*P, 144)
    else:
        patches = _im2col_batch(x)  # (B, H, W, 27)
        # block to the supertile layout: (H, W, .) -> (n_st, dh, dw, g, .)
        # h = s*8 + dh ; w = dw*12 + g ; partition p = dh*16 + dw
        p_blk = patches.reshape(B, N_ST, DH, DW, G, KK).reshape(B, N_ST * P, PFREE)
    f_blk = f.reshape(B, N_ST * P, FFREE)  # pure reshape: row-major slabs
    return [
        {"f_in": np.ascontiguousarray(f_blk[c]), "p_in": np.ascontiguousarray(p_blk[c])}
        for c in range(N_CORES)
    ]


def kernel(x: np.ndarray, f: np.ndarray) -> np.ndarray:
    import concourse.bass_utils as bass_utils

    nc = _get_nc()  # before staging: a v2->v1 fallback switches p_in layout
    in_maps = prepare_in_maps(x, f)
    res = bass_utils.run_bass_kernel_spmd(nc, in_maps, core_ids=list(range(N_CORES)))
    global LAST_RESULTS
    LAST_RESULTS = res

    out = np.empty((B, H, W, CO), dtype=np.float32)
    for c in range(N_CORES):
        o = res.results[c]["o_out"]  # (N_ST*P, OFREE)
        out[c] = o.reshape(H, W, CO)
    return out
